# revision 8
# baseline (speedup 1.0000x reference)
"""NodeClsPooler: out = x[first_node_of_each_graph] @ W.T + b, on 8 NeuronCores.

Contract: kernel(**inputs) takes FULL inputs (x [1048576,128] f32, batch
[1048576] int, W [128,128] f32, b [128] f32) and returns the FULL [8192,128]
f32 output.

Strategy (data-parallel over graphs, 1024 graphs per core):
  host: first-node index per graph via searchsorted on the sorted batch
    vector, gather those 8192 rows (the only part of x the op reads),
    transpose to channel-major, cast to bf16 (tolerance 2e-2; bf16 keeps
    rel err ~3e-3), and pack per core ONE combined input [128, 1154] bf16:
    cols [0:128] = W^T, cols [128:1152] = pooled_t shard,
    cols [1152:1154] = f32 bias bit-pattern.
  device (raw Bass, hand-scheduled, single basic block, no nc.Block):
    - input DMA partition-split across the two HWDGE queues (sync: rows
      0:64, scalar: rows 64:128) — one transfer per queue, minimal
      descriptor count; completion waits overlap the runtime preamble
    - tensor: 2 bf16 matmul chunks of 512 cols (one PSUM bank each)
    - vector: PSUM->SBUF copy + per-partition f32 bias add, bf16 out
    - out DMA [128,1024] bf16 partition-split on sync/scalar, no
      completion waits (the runtime drains the DMA queues at NEFF exit)
    - gpsimd: nothing (no SWDGE)
    - bass's 4 const-AP MEMSETs are suppressed (dead code for this kernel)
  host: concat core outputs, transpose, cast back to f32.
"""

import numpy as np
import ml_dtypes


def _enable_ldw_dedup():
    """Flip walrus's --enable-ldw-opt to true: both matmul chunks share the
    same stationary weights, and deduping the second LDWEIGHTS moves the
    first matmul ~140ns earlier on the critical chain."""
    import concourse.bass_utils as bu

    if getattr(bu, "_ldw_opt_patched", False):
        return
    _orig_run = bu.run_command

    def _patched_run(cmd, **kw):
        if isinstance(cmd, list):
            cmd = [
                "--enable-ldw-opt=true" if c == "--enable-ldw-opt=false" else c
                for c in cmd
            ]
        return _orig_run(cmd, **kw)

    bu.run_command = _patched_run
    bu._ldw_opt_patched = True


_enable_ldw_dedup()

NUM_GRAPHS = 8192
C = 128
N_CORES = 8
G_PER = NUM_GRAPHS // N_CORES  # 1024
COLS = C + G_PER + 2  # wt | pt | bias | pad  -> 1154 cols bf16 = 2308 B/partition
HALF = 512

_CACHE: dict = {}


def _build_program():
    import contextlib

    import concourse.bass as bass
    import concourse.mybir as mybir

    f32 = mybir.dt.float32
    bf16 = mybir.dt.bfloat16

    # Suppress the 4 const-AP MEMSETs bass emits at construction: this kernel
    # never reads the const APs, and dropping them removes the only dead
    # instructions in the program's startup path. memset lives on
    # BassEitherVectorEngine (the shared-interface copy in the MRO).
    _memset_cls = next(
        cls
        for cls in type(bass.Bass(target_bir_lowering=False, debug=False).gpsimd).__mro__
        if "memset" in vars(cls)
    )
    _orig_memset = _memset_cls.memset
    _memset_cls.memset = lambda self, ap, constant: None
    try:
        nc = bass.Bass(target_bir_lowering=False, debug=False)
    finally:
        _memset_cls.memset = _orig_memset

    inb = nc.dram_tensor("inb", [C, COLS], bf16, kind="ExternalInput").ap()
    outb = nc.dram_tensor("outb", [C, G_PER], bf16, kind="ExternalOutput").ap()

    with contextlib.ExitStack() as es:
        sem = {
            n: es.enter_context(nc.semaphore(n))
            for n in ["slo", "shi", "m0", "m1", "v0", "v1", "sod"]
        }
        in_s = es.enter_context(nc.sbuf_tensor("in_s", [C, COLS], bf16)).ap()
        out_s = es.enter_context(nc.sbuf_tensor("out_s", [C, G_PER], bf16)).ap()
        p0 = es.enter_context(nc.psum_tensor("p0", [C, HALF], f32)).ap()
        p1 = es.enter_context(nc.psum_tensor("p1", [C, HALF], f32)).ap()

        wt = in_s[:, 0:C]
        pt = in_s[:, C : C + G_PER]
        # bias rides in the combined bf16 buffer as an f32 bit-pattern
        # occupying two bf16 columns; DVE reads it back as [128,1] f32
        bcol = in_s[:, C + G_PER : C + G_PER + 2].bitcast(f32)

        # Hand-rolled single-bb program (no nc.Block): engines end their
        # streams without the block-exit branch/drain/barrier — the runtime's
        # own entry barrier before its semaphore-reset epilogue provides the
        # final all-engine sync, and the runtime drains the DMA queues.
        nc.sync.dma_start(out=in_s[0:64, :], in_=inb[0:64, :]).then_inc(
            sem["slo"], 16
        )
        nc.scalar.dma_start(out=in_s[64:, :], in_=inb[64:, :]).then_inc(
            sem["shi"], 16
        )

        nc.tensor.wait_ge(sem["slo"], 16)
        nc.tensor.wait_ge(sem["shi"], 16)
        nc.tensor.matmul(p0, wt, pt[:, 0:HALF], start=True, stop=True).then_inc(
            sem["m0"], 1
        )
        nc.tensor.matmul(p1, wt, pt[:, HALF:], start=True, stop=True).then_inc(
            sem["m1"], 1
        )

        nc.vector.wait_ge(sem["m0"], 1)
        nc.vector.tensor_scalar_add(out_s[:, 0:HALF], p0, bcol).then_inc(
            sem["v0"], 1
        )
        nc.vector.wait_ge(sem["m1"], 1)
        nc.vector.tensor_scalar_add(out_s[:, HALF:], p1, bcol).then_inc(
            sem["v1"], 1
        )

        # v1 alone orders both DVE chunks (in-order retirement on one engine).
        # Each queue's out split 2x32 rows: same total issue time, but only
        # the last transfer's 32 descriptors are outstanding at the runtime's
        # DGE drain.
        nc.sync.wait_ge(sem["v1"], 1)
        nc.sync.dma_start(out=outb[0:32, :], in_=out_s[0:32, :]).then_inc(
            sem["sod"], 16
        )
        nc.sync.dma_start(out=outb[32:64, :], in_=out_s[32:64, :]).then_inc(
            sem["sod"], 16
        )
        nc.scalar.wait_ge(sem["v1"], 1)
        nc.scalar.dma_start(out=outb[64:96, :], in_=out_s[64:96, :]).then_inc(
            sem["sod"], 16
        )
        nc.scalar.dma_start(out=outb[96:, :], in_=out_s[96:, :]).then_inc(
            sem["sod"], 16
        )

    return nc


def _get_program():
    if "nc" not in _CACHE:
        _CACHE["nc"] = _build_program()
    return _CACHE["nc"]


def kernel(x, batch, W, b, _trace=False, _trace_kwargs=None):
    from concourse.bass_utils import run_bass_kernel_spmd

    x = np.asarray(x)
    batch = np.asarray(batch)

    # First occurrence of each graph id in the sorted batch vector (== jnp.
    # searchsorted side='left'); clamp like jnp gather does for out-of-range.
    first = np.searchsorted(batch, np.arange(NUM_GRAPHS, dtype=batch.dtype))
    first = np.minimum(first, x.shape[0] - 1)
    pt16 = np.asarray(x[first].T, dtype=np.float32).astype(ml_dtypes.bfloat16)
    wt16 = np.asarray(W, dtype=np.float32).T.astype(ml_dtypes.bfloat16)
    b32 = np.ascontiguousarray(np.asarray(b, dtype=np.float32).reshape(C, 1))
    b_as_bf16 = b32.view(ml_dtypes.bfloat16)  # [C,2] bit-pattern of f32 bias

    in_maps = []
    for k in range(N_CORES):
        arr = np.zeros((C, COLS), dtype=ml_dtypes.bfloat16)
        arr[:, 0:C] = wt16
        arr[:, C : C + G_PER] = pt16[:, k * G_PER : (k + 1) * G_PER]
        arr[:, C + G_PER : C + G_PER + 2] = b_as_bf16
        in_maps.append({"inb": arr})

    nc = _get_program()
    res = run_bass_kernel_spmd(
        nc, in_maps, list(range(N_CORES)),
        trace=_trace, **(_trace_kwargs or {}),
    )
    out_t = np.concatenate(
        [np.asarray(res.results[k]["outb"]) for k in range(N_CORES)], axis=1
    )  # [C, NUM_GRAPHS] bf16
    out = np.ascontiguousarray(out_t.T).astype(np.float32)
    if _trace:
        _CACHE["last_results"] = res
    return out


# revision 9
# speedup vs baseline: 1.0683x; 1.0683x over previous
"""NodeClsPooler: out = x[first_node_of_each_graph] @ W.T + b, on 8 NeuronCores.

Contract: kernel(**inputs) takes FULL inputs (x [1048576,128] f32, batch
[1048576] int, W [128,128] f32, b [128] f32) and returns the FULL [8192,128]
f32 output.

Strategy (data-parallel over graphs, 1024 graphs per core):
  host: first-node index per graph via searchsorted on the sorted batch
    vector, gather those 8192 rows (the only part of x the op reads),
    transpose to channel-major, cast to bf16 (tolerance 2e-2; bf16 keeps
    rel err ~3e-3), and pack per core ONE combined input [128, 1154] bf16:
    cols [0:128] = W^T, cols [128:1152] = pooled_t shard,
    cols [1152:1154] = f32 bias bit-pattern.
  device (raw Bass, hand-scheduled, single basic block, no nc.Block):
    - input DMA partition-split across the two HWDGE queues (sync: rows
      0:64, scalar: rows 64:128) — one transfer per queue, minimal
      descriptor count; completion waits overlap the runtime preamble
    - tensor: 2 bf16 matmul chunks of 512 cols (one PSUM bank each)
    - vector: PSUM->SBUF copy + per-partition f32 bias add, bf16 out
    - out DMA [128,1024] bf16 partition-split on sync/scalar, no
      completion waits (the runtime drains the DMA queues at NEFF exit)
    - gpsimd: nothing (no SWDGE)
    - bass's 4 const-AP MEMSETs are suppressed (dead code for this kernel)
  host: concat core outputs, transpose, cast back to f32.
"""

import numpy as np
import ml_dtypes


def _enable_ldw_dedup():
    """Flip walrus's --enable-ldw-opt to true: both matmul chunks share the
    same stationary weights, and deduping the second LDWEIGHTS moves the
    first matmul ~140ns earlier on the critical chain."""
    import concourse.bass_utils as bu

    if getattr(bu, "_ldw_opt_patched", False):
        return
    _orig_run = bu.run_command

    def _patched_run(cmd, **kw):
        if isinstance(cmd, list):
            cmd = [
                "--enable-ldw-opt=true" if c == "--enable-ldw-opt=false" else c
                for c in cmd
            ]
        return _orig_run(cmd, **kw)

    bu.run_command = _patched_run
    bu._ldw_opt_patched = True


_enable_ldw_dedup()

NUM_GRAPHS = 8192
C = 128
N_CORES = 8
G_PER = NUM_GRAPHS // N_CORES  # 1024
COLS = C + G_PER + 2  # wt | pt | bias | pad  -> 1154 cols bf16 = 2308 B/partition
HALF = 512

_CACHE: dict = {}


def _build_program():
    import contextlib

    import concourse.bass as bass
    import concourse.mybir as mybir

    f32 = mybir.dt.float32
    bf16 = mybir.dt.bfloat16

    # Suppress the 4 const-AP MEMSETs bass emits at construction: this kernel
    # never reads the const APs, and dropping them removes the only dead
    # instructions in the program's startup path. memset lives on
    # BassEitherVectorEngine (the shared-interface copy in the MRO).
    _memset_cls = next(
        cls
        for cls in type(bass.Bass(target_bir_lowering=False, debug=False).gpsimd).__mro__
        if "memset" in vars(cls)
    )
    _orig_memset = _memset_cls.memset
    _memset_cls.memset = lambda self, ap, constant: None
    try:
        nc = bass.Bass(target_bir_lowering=False, debug=False)
    finally:
        _memset_cls.memset = _orig_memset

    inb = nc.dram_tensor("inb", [C, COLS], bf16, kind="ExternalInput").ap()
    outb = nc.dram_tensor("outb", [C, G_PER], bf16, kind="ExternalOutput").ap()

    with contextlib.ExitStack() as es:
        sem = {
            n: es.enter_context(nc.semaphore(n))
            for n in ["slo", "shi", "m0", "m1", "v0", "v1", "sod"]
        }
        in_s = es.enter_context(nc.sbuf_tensor("in_s", [C, COLS], bf16)).ap()
        out_s = es.enter_context(nc.sbuf_tensor("out_s", [C, G_PER], bf16)).ap()
        p0 = es.enter_context(nc.psum_tensor("p0", [C, HALF], f32)).ap()
        p1 = es.enter_context(nc.psum_tensor("p1", [C, HALF], f32)).ap()

        wt = in_s[:, 0:C]
        pt = in_s[:, C : C + G_PER]
        # bias rides in the combined bf16 buffer as an f32 bit-pattern
        # occupying two bf16 columns; DVE reads it back as [128,1] f32
        bcol = in_s[:, C + G_PER : C + G_PER + 2].bitcast(f32)

        # Hand-rolled single-bb program (no nc.Block): engines end their
        # streams without the block-exit branch/drain/barrier — the runtime's
        # own entry barrier before its semaphore-reset epilogue provides the
        # final all-engine sync, and the runtime drains the DMA queues.
        nc.sync.dma_start(out=in_s[0:64, :], in_=inb[0:64, :]).then_inc(
            sem["slo"], 16
        )
        nc.scalar.dma_start(out=in_s[64:, :], in_=inb[64:, :]).then_inc(
            sem["shi"], 16
        )

        nc.tensor.wait_ge(sem["slo"], 16)
        nc.tensor.wait_ge(sem["shi"], 16)
        nc.tensor.matmul(p0, wt, pt[:, 0:HALF], start=True, stop=True).then_inc(
            sem["m0"], 1
        )
        nc.tensor.matmul(p1, wt, pt[:, HALF:], start=True, stop=True).then_inc(
            sem["m1"], 1
        )

        nc.vector.wait_ge(sem["m0"], 1)
        nc.vector.tensor_scalar_add(out_s[:, 0:HALF], p0, bcol).then_inc(
            sem["v0"], 1
        )
        nc.vector.wait_ge(sem["m1"], 1)
        nc.vector.tensor_scalar_add(out_s[:, HALF:], p1, bcol).then_inc(
            sem["v1"], 1
        )

        # v1 alone orders both DVE chunks (in-order retirement on one engine)
        nc.sync.wait_ge(sem["v1"], 1)
        nc.sync.dma_start(out=outb[0:64, :], in_=out_s[0:64, :]).then_inc(
            sem["sod"], 16
        )
        nc.scalar.wait_ge(sem["v1"], 1)
        nc.scalar.dma_start(out=outb[64:, :], in_=out_s[64:, :]).then_inc(
            sem["sod"], 16
        )

    return nc


def _get_program():
    if "nc" not in _CACHE:
        _CACHE["nc"] = _build_program()
    return _CACHE["nc"]


def kernel(x, batch, W, b, _trace=False, _trace_kwargs=None):
    from concourse.bass_utils import run_bass_kernel_spmd

    x = np.asarray(x)
    batch = np.asarray(batch)

    # First occurrence of each graph id in the sorted batch vector (== jnp.
    # searchsorted side='left'); clamp like jnp gather does for out-of-range.
    first = np.searchsorted(batch, np.arange(NUM_GRAPHS, dtype=batch.dtype))
    first = np.minimum(first, x.shape[0] - 1)
    pt16 = np.asarray(x[first].T, dtype=np.float32).astype(ml_dtypes.bfloat16)
    wt16 = np.asarray(W, dtype=np.float32).T.astype(ml_dtypes.bfloat16)
    b32 = np.ascontiguousarray(np.asarray(b, dtype=np.float32).reshape(C, 1))
    b_as_bf16 = b32.view(ml_dtypes.bfloat16)  # [C,2] bit-pattern of f32 bias

    in_maps = []
    for k in range(N_CORES):
        arr = np.zeros((C, COLS), dtype=ml_dtypes.bfloat16)
        arr[:, 0:C] = wt16
        arr[:, C : C + G_PER] = pt16[:, k * G_PER : (k + 1) * G_PER]
        arr[:, C + G_PER : C + G_PER + 2] = b_as_bf16
        in_maps.append({"inb": arr})

    nc = _get_program()
    res = run_bass_kernel_spmd(
        nc, in_maps, list(range(N_CORES)),
        trace=_trace, **(_trace_kwargs or {}),
    )
    out_t = np.concatenate(
        [np.asarray(res.results[k]["outb"]) for k in range(N_CORES)], axis=1
    )  # [C, NUM_GRAPHS] bf16
    out = np.ascontiguousarray(out_t.T).astype(np.float32)
    if _trace:
        _CACHE["last_results"] = res
    return out
